# revision 2
# baseline (speedup 1.0000x reference)
"""Trainium2 Bass kernel v2 for nn_Attention (batch-sharded, 1 batch/core).

Math per core:
  Ktilde[a,k] = sum_b mt[b,a] * E^T[b,k]        (mt = Wk @ Wq^T)
  V[k,h]      = sum_b E^T[b,k] * Wv[b,h]        (bv added host-side post)
  S^T[k,q]*32 = sum_a Ktilde[a,:].T X^T[a,q] (+ e5m2 mask add, pre-scaled)
  P^T = exp(S^T/32 + kbias)                     (kbias host-computed: bq-terms)
  O[q,h] = P^T.T @ V / den;  den from a ones-column N=1 matmul in the O psum
Dropped exactly (softmax-invariant): bk*Q term. bv added on host.
"""
import sys
import os

sys.path.insert(0, "/opt/trn_rl_repo")
from contextlib import ExitStack

import numpy as np

import concourse.bass as bass
import concourse.tile as tile
from concourse import bacc, mybir

F32 = mybir.dt.float32
FP16 = mybir.dt.float16
E5 = mybir.dt.float8e5

B, TQ, TK, F, H = 8, 2048, 2048, 1024, 1024
N_CORES = 8
BT = 8               # contraction tiles (1024/128) for projections
A8 = 8               # a-tiles for S contraction
KT = TK // 128       # 16
QC = TQ // 512       # 4
HC = H // 512        # 2
MASK_ADD = -3200.0   # additive mask pre-scaled by sqrt(H)=32
SCALE = 1.0 / 32.0
DEN_MODE = "on"


def build_nc(iters: int = 1, den: str | None = None):
    global DEN_MODE
    if den is not None:
        DEN_MODE = den
    nc = bacc.Bacc("TRN2", target_bir_lowering=False, debug=False,
                   num_devices=N_CORES)
    xq_d = nc.dram_tensor("xq_t", [F, TQ], FP16, kind="ExternalInput").ap()
    xe_d = nc.dram_tensor("xe_t", [F, TK], FP16, kind="ExternalInput").ap()
    mt_d = nc.dram_tensor("mt", [F, F], FP16, kind="ExternalInput").ap()
    wv_d = nc.dram_tensor("wv", [F, H], FP16, kind="ExternalInput").ap()
    kb_d = nc.dram_tensor("kb", [128, KT], F32, kind="ExternalInput").ap()
    mk_d = nc.dram_tensor("maskt", [TK, TQ], E5, kind="ExternalInput").ap()
    o_d = nc.dram_tensor("o", [TQ, H], FP16, kind="ExternalOutput").ap()

    with tile.TileContext(nc) as tc, ExitStack() as ctx:
        glob = ctx.enter_context(tc.tile_pool(name="glob", bufs=1))
        pp = ctx.enter_context(tc.tile_pool(name="pp", bufs=3, space="PSUM"))
        po = ctx.enter_context(tc.tile_pool(name="po", bufs=2, space="PSUM"))
        pd = ctx.enter_context(tc.tile_pool(name="pd", bufs=1, space="PSUM"))

        # ---- persistent tensors ----
        kt_sb = glob.tile([128, A8 * TK], FP16)      # 32KB/part Ktilde^T [a][k]
        v_sb = glob.tile([128, KT * H], FP16)        # 32KB/part V [k][h]
        kb_sb = glob.tile([128, KT], F32)
        ones_col = glob.tile([128, 1], FP16)
        nc.vector.memset(ones_col[:], 1.0)

        loop_cm = tc.For_i(0, iters, 1) if iters > 1 else None
        if loop_cm is not None:
            loop_cm.__enter__()

        nc.sync.dma_start(out=kb_sb[:], in_=kb_d)

        with ExitStack() as actx:
            pha = actx.enter_context(tc.tile_pool(name="pha", bufs=1))
            xe_sb = pha.tile([128, BT * TK], FP16)   # 32KB/part E^T
            mt_sb = pha.tile([128, BT * F], FP16)    # 16KB/part mt [b][a]
            wv_sb = pha.tile([128, BT * H], FP16)    # 16KB/part Wv
            xe_rs = xe_sb[:].rearrange("p (t j) -> p t j", t=BT)
            xe_rd = xe_d.rearrange("(t p) j -> p t j", p=128)
            mt_rs = mt_sb[:].rearrange("p (t j) -> p t j", t=BT)
            mt_rd = mt_d.rearrange("(t p) j -> p t j", p=128)
            wv_rs = wv_sb[:].rearrange("p (t j) -> p t j", t=BT)
            wv_rd = wv_d.rearrange("(t p) j -> p t j", p=128)
            # fine-grained startup: mt in per-b-tile pieces on sync queue;
            # xe per kc-chunk on gpsimd queue; wv behind mt on sync
            for b_ in range(BT):
                nc.sync.dma_start(out=mt_rs[:, b_, :], in_=mt_rd[:, b_, :])
            nc.gpsimd.dma_start(out=xe_rs[:, :, 0:512], in_=xe_rd[:, :, 0:512])
            for b_ in range(BT):
                nc.sync.dma_start(out=wv_rs[:, b_, :], in_=wv_rd[:, b_, :])
            for kc in range(1, QC):
                nc.gpsimd.dma_start(out=xe_rs[:, :, kc * 512:(kc + 1) * 512],
                                    in_=xe_rd[:, :, kc * 512:(kc + 1) * 512])

            for kc in range(QC):
                # Ktilde^T[a, kc-chunk] = sum_b mt[b, a-tile].T @ E^T[b, chunk]
                for a in range(A8):
                    ps_kt = pp.tile([128, 512], F32, name="ps_kt", tag="pp")
                    for b_ in range(BT):
                        nc.tensor.matmul(
                            ps_kt[:],
                            mt_sb[:, b_ * F + a * 128:b_ * F + a * 128 + 128],
                            xe_sb[:, b_ * TK + kc * 512:b_ * TK + kc * 512 + 512],
                            start=(b_ == 0), stop=(b_ == BT - 1))
                    nc.scalar.activation(
                        kt_sb[:, a * TK + kc * 512:a * TK + kc * 512 + 512],
                        ps_kt[:], mybir.ActivationFunctionType.Identity)
                # V[k, h] = sum_b E^T[b, k-tile].T @ Wv[b, h]
                for k in range(4 * kc, 4 * kc + 4):
                    for hc in range(HC):
                        ps_v = pp.tile([128, 512], F32, name="ps_v", tag="pp")
                        for b_ in range(BT):
                            nc.tensor.matmul(
                                ps_v[:],
                                xe_sb[:, b_ * TK + k * 128:b_ * TK + k * 128 + 128],
                                wv_sb[:, b_ * H + hc * 512:b_ * H + hc * 512 + 512],
                                start=(b_ == 0), stop=(b_ == BT - 1))
                        nc.scalar.activation(
                            v_sb[:, k * H + hc * 512:k * H + hc * 512 + 512],
                            ps_v[:], mybir.ActivationFunctionType.Identity)

        # ---- per q-chunk: S^T, masked exp, O with ones-column denominator ----
        xqp = ctx.enter_context(tc.tile_pool(name="xqp", bufs=2))
        mkp = ctx.enter_context(tc.tile_pool(name="mkp", bufs=2))
        ptp = ctx.enter_context(tc.tile_pool(name="ptp", bufs=2))
        spool = ctx.enter_context(tc.tile_pool(name="spool", bufs=3))
        rpool = ctx.enter_context(tc.tile_pool(name="rpool", bufs=3))

        xq_rd = xq_d.rearrange("(t p) q -> p t q", p=128)
        mk_rd = mk_d.rearrange("(t p) q -> p t q", p=128)

        for c in range(QC):
            xqc = xqp.tile([128, A8 * 512], FP16, name="xqc", tag="xqc")
            nc.gpsimd.dma_start(
                out=xqc[:].rearrange("p (t j) -> p t j", t=A8),
                in_=xq_rd[:, :, c * 512:(c + 1) * 512])
            mkc = mkp.tile([128, KT * 512], E5, name="mkc", tag="mkc")
            nc.gpsimd.dma_start(
                out=mkc[:].rearrange("p (t j) -> p t j", t=KT),
                in_=mk_rd[:, :, c * 512:(c + 1) * 512])

            pt = ptp.tile([128, KT * 512], FP16, name="pt", tag="pt")
            for k in range(KT):
                ps_s = pp.tile([128, 512], F32, name="ps_s", tag="pp")
                for a in range(A8):
                    nc.tensor.matmul(
                        ps_s[:],
                        kt_sb[:, a * TK + k * 128:a * TK + k * 128 + 128],
                        xqc[:, a * 512:(a + 1) * 512],
                        start=(a == 0), stop=(a == A8 - 1))
                nc.vector.tensor_add(ps_s[:], ps_s[:],
                                     mkc[:, k * 512:(k + 1) * 512])
                nc.scalar.activation(pt[:, k * 512:(k + 1) * 512], ps_s[:],
                                     mybir.ActivationFunctionType.Exp,
                                     scale=SCALE, bias=kb_sb[:, k:k + 1])

            # O[q, h] += P^T[k, qsub].T @ [V | 1]; den in its own psum bank
            for qs in range(4):
                ps_o = po.tile([128, 1024], F32, name="ps_o", tag="po")
                ps_d = pd.tile([128, 1], F32, name="ps_d", tag="pd")
                for k in range(KT):
                    lhs = pt[:, k * 512 + qs * 128:k * 512 + (qs + 1) * 128]
                    nc.tensor.matmul(ps_o[:, 0:512], lhs,
                                     v_sb[:, k * H:k * H + 512],
                                     start=(k == 0), stop=(k == KT - 1))
                    nc.tensor.matmul(ps_o[:, 512:1024], lhs,
                                     v_sb[:, k * H + 512:k * H + 1024],
                                     start=(k == 0), stop=(k == KT - 1))
                    if DEN_MODE == "on":
                        nc.tensor.matmul(ps_d[:], lhs, ones_col[:],
                                         start=(k == 0), stop=(k == KT - 1))
                recip = rpool.tile([128, 1], F32, name="recip", tag="recip")
                if DEN_MODE == "on":
                    nc.vector.reciprocal(recip[:], ps_d[:])
                else:
                    nc.vector.memset(recip[:], 1.0)
                stage = spool.tile([128, 1024], FP16, name="stage")
                nc.vector.tensor_scalar_mul(stage[:], ps_o[:, 0:1024],
                                            recip[:])
                nc.sync.dma_start(
                    out=o_d[c * 512 + qs * 128:c * 512 + (qs + 1) * 128, :],
                    in_=stage[:])

        if loop_cm is not None:
            loop_cm.__exit__(None, None, None)

    nc.compile()
    return nc


# ---------------------------------------------------------------------------
# PJRT execution (axon) — self-contained runner
# ---------------------------------------------------------------------------
class SpmdRunner:
    def __init__(self, nc, n_cores=N_CORES):
        import jax
        from jax.sharding import Mesh, PartitionSpec
        from jax.experimental.shard_map import shard_map
        from concourse.bass2jax import (_bass_exec_p, install_neuronx_cc_hook,
                                        partition_id_tensor)

        install_neuronx_cc_hook()
        self.jax = jax
        self.nc = nc
        self.n_cores = n_cores
        in_names, out_names, out_avals, zero_outs = [], [], [], []
        for alloc in nc.m.functions[0].allocations:
            if not isinstance(alloc, mybir.MemoryLocationSet):
                continue
            name = alloc.memorylocations[0].name
            if alloc.kind == "ExternalInput":
                if (nc.partition_id_tensor is None
                        or name != nc.partition_id_tensor.name):
                    in_names.append(name)
            elif alloc.kind == "ExternalOutput":
                out_names.append(name)
                shape = tuple(alloc.tensor_shape)
                dtype = mybir.dt.np(alloc.dtype)
                out_avals.append(jax.core.ShapedArray(shape, dtype))
                zero_outs.append(np.zeros(shape, dtype))
        self.in_names, self.out_names = in_names, out_names
        self.out_avals, self.zero_outs = out_avals, zero_outs
        n_params = len(in_names)
        pname = nc.partition_id_tensor.name if nc.partition_id_tensor else None
        all_in = list(in_names) + list(out_names)
        if pname is not None:
            all_in.append(pname)

        def _body(*args):
            operands = list(args)
            if pname is not None:
                operands.append(partition_id_tensor())
            outs = _bass_exec_p.bind(
                *operands, out_avals=tuple(out_avals), in_names=tuple(all_in),
                out_names=tuple(out_names), lowering_input_output_aliases=(),
                sim_require_finite=True, sim_require_nnan=True, nc=nc)
            return tuple(outs)

        devices = jax.devices()[:n_cores]
        self.mesh = Mesh(np.asarray(devices), ("core",))
        n_outs = len(out_names)
        self.fn = jax.jit(
            shard_map(_body, mesh=self.mesh,
                      in_specs=(PartitionSpec("core"),) * (n_params + n_outs),
                      out_specs=(PartitionSpec("core"),) * n_outs,
                      check_rep=False),
            keep_unused=True)
        self._staged = None

    def stage(self, in_maps):
        from jax.sharding import NamedSharding, PartitionSpec
        n = self.n_cores
        concat = [np.concatenate([np.asarray(in_maps[c][name])
                                  for c in range(n)], axis=0)
                  for name in self.in_names]
        concat += [np.zeros((n * z.shape[0], *z.shape[1:]), z.dtype)
                   for z in self.zero_outs]
        sh = NamedSharding(self.mesh, PartitionSpec("core"))
        self._staged = [self.jax.device_put(x, sh) for x in concat]

    def run(self):
        out = self.fn(*self._staged)
        self.jax.block_until_ready(out)
        return out

    def fetch(self, out):
        res = []
        for c in range(self.n_cores):
            d = {}
            for i, name in enumerate(self.out_names):
                arr = np.asarray(out[i])
                d[name] = arr.reshape(self.n_cores, *self.out_avals[i].shape)[c]
            res.append(d)
        return res


def prep_in_maps(query, encoder_states, target_mask, Wq, bq, Wk, bk, Wv, bv):
    import ml_dtypes
    Wq64 = np.asarray(Wq, np.float64)
    Wk64 = np.asarray(Wk, np.float64)
    mt = (Wk64 @ Wq64.T).astype(np.float16)          # [b, a]
    wv16 = np.asarray(Wv, np.float16)
    u = Wk64 @ np.asarray(bq, np.float64)            # [F]
    bqbk = float(np.asarray(bq, np.float64) @ np.asarray(bk, np.float64))

    in_maps = []
    for b in range(N_CORES):
        xq_t = np.ascontiguousarray(np.asarray(query[b]).T.astype(np.float16))
        e64 = np.asarray(encoder_states[b], np.float64)
        xe_t = np.ascontiguousarray(e64.T.astype(np.float16))
        kb = ((e64 @ u) + bqbk) * (1.0 / 32.0)       # [TK]
        kb_r = np.ascontiguousarray(
            kb.reshape(KT, 128).T.astype(np.float32))  # [128, KT]
        maskt = np.where(np.asarray(target_mask[b]).T, np.float32(0),
                         np.float32(MASK_ADD)).astype(ml_dtypes.float8_e5m2)
        in_maps.append({
            "xq_t": xq_t, "xe_t": xe_t, "mt": mt, "wv": wv16,
            "kb": kb_r, "maskt": np.ascontiguousarray(maskt),
        })
    return in_maps


_RUNNER_CACHE = {}


def get_runner(iters: int = 1):
    if iters not in _RUNNER_CACHE:
        nc = build_nc(iters)
        _RUNNER_CACHE[iters] = SpmdRunner(nc)
    return _RUNNER_CACHE[iters]


def kernel(query, encoder_states, target_mask, Wq, bq, Wk, bk, Wv, bv):
    r = get_runner(1)
    r.stage(prep_in_maps(query, encoder_states, target_mask,
                         Wq, bq, Wk, bk, Wv, bv))
    res = r.fetch(r.run())
    out = np.stack([res[b]["o"] for b in range(N_CORES)]).astype(np.float32)
    return out + np.asarray(bv, np.float32)[None, None, :]
